# revision 21
# baseline (speedup 1.0000x reference)
"""Trainium2 Bass kernel for nn_CSG_layer (CSG layer: latent-conditioned softmax
mixing + gumbel routing + batched GEMM + tiny MLP side output).

Contract: kernel(**inputs) takes FULL unsharded inputs (as produced by
setup_inputs) and returns the full outputs (y, v_encode). Internally shards
batch dim 32 across 8 NeuronCores (4 batches/core), runs one SPMD Bass
program, and gathers.

Self-contained: only imports environment-provided packages (numpy, concourse).
"""
import os
import numpy as np
from contextlib import ExitStack

import concourse.bass as bass
import concourse.bacc as bacc
import concourse.tile as tile
from concourse import mybir
from concourse.bass_utils import run_bass_kernel_spmd
from concourse.masks import make_identity

F32 = mybir.dt.float32
BF16 = mybir.dt.bfloat16
EPS = float(np.finfo(np.float32).eps)

# Problem shape constants (fixed by the problem spec).
B, P, S_IN, S_OUT, L = 32, 8192, 64, 32, 256
N_CORES = 8
BPC = B // N_CORES          # batches per core = 4
Q = 2 * S_OUT * S_IN        # 4096 flattened (side, o, i)

# Main-GEMM compute dtype: "f32" (exact) or "bf16" (fast).
MM_DTYPE = os.environ.get("CSG_MM_DTYPE", "bf16")
# Logits matmul dtype: "f32" or "f32r"
LOGITS_DTYPE = os.environ.get("CSG_LOGITS_DTYPE", "f32r")

# Stash of the last BassKernelResults (for test.py profiling access).
LAST_RESULTS = None

_PROGRAM_CACHE = {}


def _build_program(p_exp: float):
    """Build the SPMD Bass program (same on every core)."""
    nc = bacc.Bacc()

    x_mm_dt = BF16 if MM_DTYPE == "bf16" else F32

    # ---- DRAM I/O (per-core shard shapes) ----
    logit_dt = mybir.dt.float32r if LOGITS_DTYPE == "f32r" else F32
    xT_d = nc.dram_tensor("xT", [BPC, S_IN, P], x_mm_dt, kind="ExternalInput")
    kcat_d = nc.dram_tensor("kcat", [L, Q], logit_dt, kind="ExternalInput")
    latT_d = nc.dram_tensor("latT", [L, BPC], logit_dt, kind="ExternalInput")
    guT_d = nc.dram_tensor("guT", [S_IN, 2, S_OUT, BPC], F32, kind="ExternalInput")
    w1_d = nc.dram_tensor("w1", [Q, L], F32, kind="ExternalInput")
    w2_d = nc.dram_tensor("w2", [L, L], F32, kind="ExternalInput")
    b1_d = nc.dram_tensor("b1v", [L], F32, kind="ExternalInput")
    b2_d = nc.dram_tensor("b2v", [L], F32, kind="ExternalInput")
    y_d = nc.dram_tensor("y", [BPC, P, 4 * S_OUT], F32, kind="ExternalOutput")
    ve_d = nc.dram_tensor("ve", [L, BPC], F32, kind="ExternalOutput")

    AF = mybir.ActivationFunctionType
    OP = mybir.AluOpType

    with tile.TileContext(nc) as tc, ExitStack() as ctx:
        consts = ctx.enter_context(tc.tile_pool(name="consts", bufs=1))
        sb = ctx.enter_context(tc.tile_pool(name="sb", bufs=1))

        ident = consts.tile([128, 128], F32)
        make_identity(nc, ident)
        ones64 = consts.tile([64, 1], F32)
        nc.vector.memset(ones64, 1.0)
        eps_sb = consts.tile([128, 1], F32)
        nc.vector.memset(eps_sb, EPS)

        # ---- small input DMAs ----
        lat_sb = sb.tile([128, 2, BPC], logit_dt)
        nc.sync.dma_start(lat_sb, latT_d.rearrange("(c p) b -> p c b", p=128))
        kcat_sb = sb.tile([128, 2, Q], logit_dt)
        nc.sync.dma_start(kcat_sb, kcat_d.rearrange("(c p) q -> p c q", p=128))
        gu_sb = sb.tile([S_IN, 2, S_OUT, BPC], F32)
        nc.sync.dma_start(gu_sb, guT_d[:])
        w1_sb = sb.tile([128, Q // 128, L], F32)
        nc.sync.dma_start(w1_sb, w1_d.rearrange("(c p) j -> p c j", p=128))
        w2_sb = sb.tile([128, 2, L], F32)
        nc.sync.dma_start(w2_sb, w2_d.rearrange("(c p) j -> p c j", p=128))
        b1_sb = sb.tile([128, 2], F32)
        nc.sync.dma_start(b1_sb, b1_d.rearrange("(c p) -> p c", p=128))
        b2_sb = sb.tile([128, 2], F32)
        nc.sync.dma_start(b2_sb, b2_d.rearrange("(c p) -> p c", p=128))

        # ---- P1/P2: logits -> E = exp(logits)  (layout [BPC, (side,o,i)]) ----
        E_sb = sb.tile([BPC, Q], F32)
        lat_mm = lat_sb[:]
        kcat_mm = kcat_sb[:]
        with tc.tile_pool(name="ps_log", bufs=2, space="PSUM") as ps_log:
            for qc in range(4):  # chunks of 1024
                pt_log = ps_log.tile([BPC, 1024], F32)
                for half in range(2):
                    lo = qc * 1024 + half * 512
                    for lc in range(2):
                        nc.tensor.matmul(
                            pt_log[:, half * 512:(half + 1) * 512],
                            lat_mm[:, lc, :],
                            kcat_mm[:, lc, lo:lo + 512],
                            start=(lc == 0),
                            stop=(lc == 1),
                        )
                nc.scalar.activation(
                    E_sb[:, qc * 1024:(qc + 1) * 1024], pt_log[:], AF.Exp
                )

        # S[b, g] = sum_i E ; A = E + EPS*S ; lnA = ln(A) ; V = E/S
        S_sb = sb.tile([BPC, 64], F32)
        nc.vector.tensor_reduce(
            S_sb[:], E_sb[:].rearrange("p (g i) -> p g i", i=S_IN),
            axis=mybir.AxisListType.X, op=OP.add,
        )
        A_sb = sb.tile([BPC, Q], F32)
        nc.vector.scalar_tensor_tensor(
            out=A_sb[:].rearrange("p (g i) -> p g i", i=S_IN),
            in0=S_sb[:, :, None].broadcast_to([BPC, 64, S_IN]),
            scalar=EPS,
            in1=E_sb[:].rearrange("p (g i) -> p g i", i=S_IN),
            op0=OP.mult,
            op1=OP.add,
        )
        lnA_sb = sb.tile([BPC, Q], F32)
        nc.scalar.activation(lnA_sb[:], A_sb[:], AF.Ln)
        recipS = sb.tile([BPC, 64], F32)
        nc.vector.reciprocal(recipS[:], S_sb[:])
        V_sb = sb.tile([BPC, Q], F32)
        nc.vector.tensor_mul(
            V_sb[:].rearrange("p (g i) -> p g i", i=S_IN),
            E_sb[:].rearrange("p (g i) -> p g i", i=S_IN),
            recipS[:, :, None].broadcast_to([BPC, 64, S_IN]),
        )

        # ---- P3: PE transposes ----
        # lnAT[i, side, o, b] ; VT[kpart, kc, b]
        lnAT = sb.tile([S_IN, 2, S_OUT, BPC], F32)
        VT = sb.tile([128, Q // 128, BPC], F32)
        with tc.tile_pool(name="ps_tr", bufs=4, space="PSUM") as ps_tr:
            for g in range(64):  # g = side*32 + o
                pt = ps_tr.tile([128, BPC], F32)
                nc.tensor.transpose(
                    pt[:S_IN, :], lnA_sb[:, g * S_IN:(g + 1) * S_IN],
                    ident[:BPC, :BPC],
                )
                nc.any.tensor_copy(lnAT[:, g // 32, g % 32, :], pt[:S_IN, :])
            for kc in range(Q // 128):
                pt = ps_tr.tile([128, BPC], F32)
                nc.tensor.transpose(
                    pt[:], V_sb[:, kc * 128:(kc + 1) * 128], ident[:BPC, :BPC]
                )
                nc.any.tensor_copy(VT[:, kc, :], pt[:])

            # ---- P4: gumbel chain in T layout [S_IN, (side,o,b)] ----
            NF = 2 * S_OUT * BPC  # 256
            uc = sb.tile([S_IN, NF], F32)
            nc.vector.tensor_scalar_max(uc[:], gu_sb[:].rearrange("p a b c -> p (a b c)"), EPS)
            lnu = sb.tile([S_IN, NF], F32)
            nc.scalar.activation(lnu[:], uc[:], AF.Ln)
            lnB = sb.tile([S_IN, NF], F32)
            nc.scalar.activation(
                lnB[:], lnu[:], AF.Ln, scale=-1.0, bias=eps_sb[:S_IN, :]
            )
            dch = sb.tile([S_IN, NF], F32)
            nc.vector.tensor_sub(
                dch[:], lnAT[:].rearrange("p a b c -> p (a b c)"), lnB[:]
            )
            ngum = sb.tile([S_IN, NF], F32)
            nc.scalar.activation(ngum[:], dch[:], AF.Exp, scale=float(p_exp))

            # ---- P5/P6: group sums over i (partition dim) via ones-matmul ----
            sg_ps = ps_tr.tile([1, NF], F32, tag="sg")
            nc.tensor.matmul(sg_ps[:], ones64[:], ngum[:], start=True, stop=True)
            recipG = sb.tile([1, NF], F32)
            nc.vector.reciprocal(recipG[:], sg_ps[:])
        repG = sb.tile([S_IN, NF], F32)
        nc.gpsimd.partition_broadcast(repG[:], recipG[:])
        maskT = sb.tile([S_IN, 2, S_OUT, BPC], F32)
        nc.vector.tensor_mul(
            maskT[:].rearrange("p a b c -> p (a b c)"),
            ngum[:],
            repG[:],
        )

        # ---- P7: per-batch moving operand M2 [128, (b, 2*S_OUT)] ----
        # cols 0:32 = L+R, 32:64 = L-R. Rows 0:64 built by DVE; rows 64:128
        # are a copy (SBUF->SBUF DMA) so batch-pair row-tiled matmuls can
        # stream the same mask through PE rows 64-127.
        m2_dt = BF16 if MM_DTYPE == "bf16" else F32
        M2 = sb.tile([128, BPC, 2 * S_OUT], m2_dt)
        for b in range(BPC):
            nc.vector.tensor_add(
                M2[0:S_IN, b, 0:S_OUT], maskT[:, 0, :, b], maskT[:, 1, :, b]
            )
            nc.vector.tensor_sub(
                M2[0:S_IN, b, S_OUT:2 * S_OUT], maskT[:, 0, :, b], maskT[:, 1, :, b]
            )
        nc.sync.dma_start(M2[S_IN:128, :, :], M2[0:S_IN, :, :])

        # ---- P8: main GEMM + clip + y out ----
        # per batch: 4 xa tiles of [64, 2048]; 16 chunks of 128 points each
        XCOLS = 2048
        NCH = XCOLS // 128       # 16 chunks per xa tile
        xpool = ctx.enter_context(tc.tile_pool(name="xpool", bufs=2))
        ypool = ctx.enter_context(tc.tile_pool(name="ypool", bufs=4))
        GRP = 8  # psum chunks per group (one full bank)
        with tc.tile_pool(name="ps_main", bufs=6, space="PSUM") as ps_main:
            for bp in range(BPC // 2):        # batch pairs (2bp, 2bp+1)
                for t4 in range(P // XCOLS):
                    xa = xpool.tile([128, XCOLS], x_mm_dt, tag="xa")
                    # two half-partition DMAs (different SBUF port groups)
                    nc.sync.dma_start(
                        xa[0:S_IN, :],
                        xT_d[2 * bp, :, t4 * XCOLS:(t4 + 1) * XCOLS],
                    )
                    nc.sync.dma_start(
                        xa[S_IN:128, :],
                        xT_d[2 * bp + 1, :, t4 * XCOLS:(t4 + 1) * XCOLS],
                    )
                    for half in range(NCH // GRP):
                        pms = [
                            ps_main.tile(
                                [128, GRP, 2 * S_OUT], F32, tag="pm",
                                name=f"pm{h}",
                            )
                            for h in range(2)
                        ]
                        # interleave the two row-groups so PE overlaps them
                        for k in range(GRP):
                            ck = half * GRP + k
                            for h in range(2):
                                nc.tensor.matmul(
                                    pms[h][:, k, :],
                                    xa[h * S_IN:(h + 1) * S_IN,
                                       ck * 128:(ck + 1) * 128],
                                    M2[h * S_IN:(h + 1) * S_IN, 2 * bp + h, :],
                                    start=True, stop=True,
                                )
                        for h in range(2):
                            pm = pms[h]
                            y_sb = ypool.tile([128, GRP, 4 * S_OUT], F32, tag="ysb")
                            # block0 = min(yl+yr, 1); block1 = max(yl+yr-1, 0)
                            # block2 = max(yl-yr, 0); block3 = max(yr-yl, 0)
                            nc.vector.tensor_scalar_min(
                                y_sb[:, :, 0:32], pm[:, :, 0:32], 1.0
                            )
                            nc.vector.tensor_scalar(
                                out=y_sb[:, :, 32:64], in0=pm[:, :, 0:32],
                                scalar1=1.0, scalar2=0.0,
                                op0=OP.subtract, op1=OP.max,
                            )
                            nc.scalar.activation(
                                y_sb[:, :, 64:96], pm[:, :, 32:64], AF.Relu
                            )
                            nc.scalar.activation(
                                y_sb[:, :, 96:128], pm[:, :, 32:64], AF.Relu,
                                scale=-1.0,
                            )
                            p0 = t4 * XCOLS + half * GRP * 128
                            nc.sync.dma_start(
                                y_d[2 * bp + h, p0:p0 + GRP * 128, :].rearrange(
                                    "(c p) f -> p c f", p=128
                                ),
                                y_sb[:],
                            )

            # ---- P9: MLP (v_encode) ----
            with tc.tile_pool(name="ps_mlp", bufs=2, space="PSUM") as ps_mlp:
                h_sb = sb.tile([128, 2, BPC], F32)
                for jc in range(2):
                    ph = ps_mlp.tile([128, BPC], F32, tag="mlp")
                    for kc in range(Q // 128):
                        nc.tensor.matmul(
                            ph[:],
                            w1_sb[:, kc, jc * 128:(jc + 1) * 128],
                            VT[:, kc, :],
                            start=(kc == 0), stop=(kc == Q // 128 - 1),
                        )
                    nc.scalar.activation(
                        h_sb[:, jc, :], ph[:], AF.Lrelu,
                        bias=b1_sb[:, jc:jc + 1], scale=1.0, alpha=0.01,
                    )
                ve_sb = sb.tile([128, 2, BPC], F32)
                for j2c in range(2):
                    pv = ps_mlp.tile([128, BPC], F32, tag="mlp")
                    for kc in range(2):
                        nc.tensor.matmul(
                            pv[:],
                            w2_sb[:, kc, j2c * 128:(j2c + 1) * 128],
                            h_sb[:, kc, :],
                            start=(kc == 0), stop=(kc == 1),
                        )
                    nc.scalar.activation(
                        ve_sb[:, j2c, :], pv[:], AF.Identity,
                        bias=b2_sb[:, j2c:j2c + 1],
                    )
                nc.sync.dma_start(
                    ve_d.rearrange("(c p) b -> p c b", p=128), ve_sb[:]
                )

    if not nc.is_finalized():
        nc.finalize()
    return nc


def kernel(x, latent_vec, gumbel_u, K_left, K_right, temp, W1, b1, W2, b2):
    global LAST_RESULTS
    x = np.ascontiguousarray(x, np.float32)
    latent_vec = np.ascontiguousarray(latent_vec, np.float32)
    gumbel_u = np.ascontiguousarray(gumbel_u, np.float32)

    # host layout prep (pure layout/dtype, no math)
    if MM_DTYPE == "bf16":
        import ml_dtypes
        xT = np.ascontiguousarray(x.transpose(0, 2, 1).astype(ml_dtypes.bfloat16))
    else:
        xT = np.ascontiguousarray(x.transpose(0, 2, 1))          # [32, 64, 8192]
    Kcat = np.ascontiguousarray(
        np.stack([K_left.transpose(0, 2, 1), K_right.transpose(0, 2, 1)], axis=1)
    ).reshape(L, Q).astype(np.float32)
    latT = np.ascontiguousarray(latent_vec.T)                     # [256, 32]
    guTall = np.ascontiguousarray(gumbel_u[:, :, 0].transpose(2, 1, 3, 0))  # [64,2,32,32]
    W1perm = np.ascontiguousarray(
        np.asarray(W1, np.float32)
        .reshape(2, S_IN, S_OUT, L).transpose(0, 2, 1, 3).reshape(Q, L)
    )
    W2c = np.ascontiguousarray(np.asarray(W2, np.float32))
    b1c = np.ascontiguousarray(np.asarray(b1, np.float32))
    b2c = np.ascontiguousarray(np.asarray(b2, np.float32))
    t_clip = float(np.clip(np.float32(np.asarray(temp).reshape(-1)[0]), EPS, 2.0))
    p_exp = 1.0 / max(t_clip, EPS)

    key = (round(p_exp, 9), MM_DTYPE, LOGITS_DTYPE)
    if key not in _PROGRAM_CACHE:
        _PROGRAM_CACHE[key] = _build_program(p_exp)
    nc = _PROGRAM_CACHE[key]

    in_maps = []
    for core in range(N_CORES):
        b0 = core * BPC
        in_maps.append({
            "xT": np.ascontiguousarray(xT[b0:b0 + BPC]),
            "kcat": Kcat,
            "latT": np.ascontiguousarray(latT[:, b0:b0 + BPC]),
            "guT": np.ascontiguousarray(guTall[..., b0:b0 + BPC]),
            "w1": W1perm,
            "w2": W2c,
            "b1v": b1c,
            "b2v": b2c,
        })

    trace = bool(int(os.environ.get("CSG_TRACE", "0")))
    res = run_bass_kernel_spmd(
        nc, in_maps, core_ids=list(range(N_CORES)), trace=trace
    )
    LAST_RESULTS = res
    y = np.concatenate([r["y"] for r in res.results], axis=0)
    ve = np.concatenate([r["ve"].T for r in res.results], axis=0)
    return y.astype(np.float32), ve.astype(np.float32)


# revision 27
# speedup vs baseline: 1.2316x; 1.2316x over previous
"""Trainium2 Bass kernel for nn_CSG_layer (CSG layer: latent-conditioned softmax
mixing + gumbel routing + batched GEMM + tiny MLP side output).

Contract: kernel(**inputs) takes FULL unsharded inputs (as produced by
setup_inputs) and returns the full outputs (y, v_encode). Internally shards
batch dim 32 across 8 NeuronCores (4 batches/core), runs one SPMD Bass
program, and gathers.

Device pipeline (per core, batches b=0..3):
  logits are computed TRANSPOSED ([q, b], 64 small matmuls with Kcat chunks
  as stationary weights), so the entire softmax + gumbel chain runs at full
  128-partition width; group sums over S_in use select-matrix matmuls and
  partition broadcasts use select^T matmuls. The main GEMM uses a
  block-diagonal batch-pair mask (K=128, N=128), clip/evacuation is split
  between VectorE and ScalarE, and the MLP reuses E_T directly (no
  transposes anywhere outside the PE-free layout plumbing).
"""
import os
import numpy as np
from contextlib import ExitStack

import concourse.bass as bass
import concourse.bacc as bacc
import concourse.tile as tile
from concourse import mybir
from concourse.bass_utils import run_bass_kernel_spmd

F32 = mybir.dt.float32
BF16 = mybir.dt.bfloat16
EPS = float(np.finfo(np.float32).eps)

# Problem shape constants (fixed by the problem spec).
B, P, S_IN, S_OUT, L = 32, 8192, 64, 32, 256
N_CORES = 8
BPC = B // N_CORES          # batches per core = 4
Q = 2 * S_OUT * S_IN        # 4096 flattened (side, o, i)

MM_DTYPE = os.environ.get("CSG_MM_DTYPE", "bf16")       # main GEMM: bf16|f32
LOGITS_DTYPE = os.environ.get("CSG_LOGITS_DTYPE", "f32r")  # f32r|f32
Y_DTYPE = os.environ.get("CSG_Y_DTYPE", "f32")          # y store: f32|bf16

LAST_RESULTS = None
_PROGRAM_CACHE = {}


def _build_program(p_exp: float):
    nc = bacc.Bacc()

    x_mm_dt = BF16 if MM_DTYPE == "bf16" else F32
    logit_dt = mybir.dt.float32r if LOGITS_DTYPE == "f32r" else F32
    y_dt = BF16 if Y_DTYPE == "bf16" else F32

    xT_d = nc.dram_tensor("xT", [BPC, S_IN, P], x_mm_dt, kind="ExternalInput")
    kcat_d = nc.dram_tensor("kcat", [L, Q], logit_dt, kind="ExternalInput")
    latT_d = nc.dram_tensor("latT", [L, BPC], logit_dt, kind="ExternalInput")
    gu_d = nc.dram_tensor("guT", [128, 32, BPC], F32, kind="ExternalInput")
    w1_d = nc.dram_tensor("w1", [Q, L], F32, kind="ExternalInput")
    w2_d = nc.dram_tensor("w2", [L, L], F32, kind="ExternalInput")
    b1_d = nc.dram_tensor("b1v", [L], F32, kind="ExternalInput")
    b2_d = nc.dram_tensor("b2v", [L], F32, kind="ExternalInput")
    sel_d = nc.dram_tensor("selc", [128, 2], F32, kind="ExternalInput")
    selT_d = nc.dram_tensor("selTc", [2, 128], F32, kind="ExternalInput")
    y_d = nc.dram_tensor("y", [BPC, P, 4 * S_OUT], y_dt, kind="ExternalOutput")
    ve_d = nc.dram_tensor("ve", [L, BPC], F32, kind="ExternalOutput")

    AF = mybir.ActivationFunctionType
    OP = mybir.AluOpType
    NF = 32 * BPC  # 128 free elements in T2 layout (j, b)

    with tile.TileContext(nc) as tc, ExitStack() as ctx:
        consts = ctx.enter_context(tc.tile_pool(name="consts", bufs=1))
        sb = ctx.enter_context(tc.tile_pool(name="sb", bufs=1))

        # select matrices for partition-group sums / broadcasts (host-fed)
        sel = consts.tile([128, 2], F32)       # sel[p, j] = (p // 64 == j)
        nc.sync.dma_start(sel, sel_d[:])
        selT = consts.tile([2, 128], F32)      # selT[j, p] = (p // 64 == j)
        nc.sync.dma_start(selT, selT_d[:])
        eps_sb = consts.tile([128, 1], F32)
        nc.vector.memset(eps_sb, EPS)

        # ---- prologue input DMAs ----
        lat_sb = sb.tile([128, 2, BPC], logit_dt)
        nc.sync.dma_start(lat_sb, latT_d.rearrange("(c p) b -> p c b", p=128))
        gu_sb = sb.tile([128, 32, BPC], F32)
        nc.sync.dma_start(gu_sb, gu_d[:])
        kcat_sb = sb.tile([128, 2, Q], logit_dt)
        for qq in range(4):  # chunked so logits matmuls pipeline with the load
            nc.sync.dma_start(
                kcat_sb[:, :, qq * 1024:(qq + 1) * 1024],
                kcat_d.rearrange("(c p) q -> p c q", p=128)[
                    :, :, qq * 1024:(qq + 1) * 1024],
            )

        # ---- logits^T: 64 MMs -> E_T [128 (q%128), 32 (j=q//128), BPC] ----
        E_T = sb.tile([128, 32, BPC], F32)
        with tc.tile_pool(name="ps_pre", bufs=1, space="PSUM") as ps_pre:
            for qg in range(4):  # groups of 8 j-chunks
                pt_log = ps_pre.tile([128, 8, BPC], F32, tag="ptlog", bufs=2)
                for k in range(8):
                    j = qg * 8 + k
                    for lc in range(2):
                        nc.tensor.matmul(
                            pt_log[:, k, :],
                            kcat_sb[:, lc, j * 128:(j + 1) * 128],
                            lat_sb[:, lc, :],
                            start=(lc == 0), stop=(lc == 1),
                        )
                nc.scalar.activation(
                    E_T[:, qg * 8:(qg + 1) * 8, :], pt_log[:], AF.Exp
                )

            # ---- group sums over i; A = E + eps*S ; lnA ----
            ef = E_T[:].rearrange("p a b -> p (a b)")
            sg_ps = ps_pre.tile([2, NF], F32, tag="sg", bufs=2)
            nc.tensor.matmul(sg_ps[:], sel[:], ef, start=True, stop=True)
            sg_sb = sb.tile([2, NF], F32)
            nc.vector.tensor_copy(sg_sb[:], sg_ps[:])
            srep_ps = ps_pre.tile([128, NF], F32, tag="rep", bufs=2)
            nc.tensor.matmul(srep_ps[:], selT[:], sg_sb[:], start=True, stop=True)

            # reciprocal of group sums (for the MLP V input), while in psum
            rS = sb.tile([128, NF], F32)
            nc.vector.reciprocal(rS[:], srep_ps[:])

            A_T = sb.tile([128, NF], F32)
            nc.vector.scalar_tensor_tensor(
                out=A_T[:], in0=srep_ps[:], scalar=EPS, in1=ef,
                op0=OP.mult, op1=OP.add,
            )
            lnA_T = sb.tile([128, NF], F32)
            nc.scalar.activation(lnA_T[:], A_T[:], AF.Ln)

            # ---- gumbel chain (full width) ----
            guf = gu_sb[:].rearrange("p a b -> p (a b)")
            uc = sb.tile([128, NF], F32)
            nc.vector.tensor_scalar_max(uc[:], guf, EPS)
            lnu = sb.tile([128, NF], F32)
            nc.scalar.activation(lnu[:], uc[:], AF.Ln)
            lnB = sb.tile([128, NF], F32)
            nc.scalar.activation(
                lnB[:], lnu[:], AF.Ln, scale=-1.0, bias=eps_sb[:]
            )
            dch = sb.tile([128, NF], F32)
            nc.vector.tensor_sub(dch[:], lnA_T[:], lnB[:])
            ngum = sb.tile([128, NF], F32)
            nc.scalar.activation(ngum[:], dch[:], AF.Exp, scale=float(p_exp))

            sg2_ps = ps_pre.tile([2, NF], F32, tag="sg", bufs=2)
            nc.tensor.matmul(sg2_ps[:], sel[:], ngum[:], start=True, stop=True)
            rg_sb = sb.tile([2, NF], F32)
            nc.vector.reciprocal(rg_sb[:], sg2_ps[:])
            grep_ps = ps_pre.tile([128, NF], F32, tag="rep", bufs=2)
            nc.tensor.matmul(grep_ps[:], selT[:], rg_sb[:], start=True, stop=True)
            maskT2 = sb.tile([128, 32, BPC], F32)
            nc.vector.tensor_mul(
                maskT2[:].rearrange("p a b -> p (a b)"), ngum[:], grep_ps[:]
            )

            # MLP V input (off critical path, cheap): V = E/S in-place layout
            VT_mlp = sb.tile([128, 32, BPC], F32)
            nc.vector.tensor_mul(
                VT_mlp[:].rearrange("p a b -> p (a b)"), ef, rS[:]
            )

        # ---- un-interleave mask parity: maskLOW[i, side, oh, par, b] ----
        # (o = 2*oh + par, so (oh, par) lexicographic == o ascending)
        maskLOW = sb.tile([S_IN, 2, 16, 2, BPC], F32)
        mview = maskT2[:].rearrange("p (s oh) b -> p s oh b", s=2)
        for par in range(2):
            nc.sync.dma_start(
                maskLOW[:, :, :, par, :],
                mview[par * 64:(par + 1) * 64, :, :, :],
            )

        # ---- M2low [64, b, 64] (cols: L+R | L-R), bf16 for the main GEMM ----
        m2_dt = BF16 if MM_DTYPE == "bf16" else F32
        M2low = sb.tile([S_IN, BPC, 2 * S_OUT], m2_dt)
        for b in range(BPC):
            outv = M2low[:, b, :].rearrange("p (two oh par) -> p two oh par", two=2, oh=16)
            nc.vector.tensor_add(
                outv[:, 0, :, :], maskLOW[:, 0, :, :, b], maskLOW[:, 1, :, :, b]
            )
            nc.vector.tensor_sub(
                outv[:, 1, :, :], maskLOW[:, 0, :, :, b], maskLOW[:, 1, :, :, b]
            )

        # ---- block-diagonal batch-pair masks M2blk [128, pair, 128] ----
        M2blk = sb.tile([128, BPC // 2, 128], m2_dt)
        nc.vector.memset(M2blk, 0.0)
        for bp in range(BPC // 2):
            nc.vector.tensor_copy(
                M2blk[0:S_IN, bp, 0:2 * S_OUT], M2low[:, 2 * bp, :]
            )
            nc.sync.dma_start(
                M2blk[S_IN:128, bp, 2 * S_OUT:128], M2low[:, 2 * bp + 1, :]
            )

        # ---- main GEMM + clip + y out ----
        XCOLS = 2048
        NCH = XCOLS // 128       # 16 chunks per xa tile
        GRP = 4                  # chunks per psum bank
        xpool = ctx.enter_context(tc.tile_pool(name="xpool", bufs=3))
        ypool = ctx.enter_context(tc.tile_pool(name="ypool", bufs=3))
        with tc.tile_pool(name="ps_main", bufs=6, space="PSUM") as ps_main:
            for bp in range(BPC // 2):
                for t4 in range(P // XCOLS):
                    xa = xpool.tile([128, XCOLS], x_mm_dt, tag="xa")
                    nc.sync.dma_start(
                        xa[0:S_IN, :],
                        xT_d[2 * bp, :, t4 * XCOLS:(t4 + 1) * XCOLS],
                    )
                    nc.sync.dma_start(
                        xa[S_IN:128, :],
                        xT_d[2 * bp + 1, :, t4 * XCOLS:(t4 + 1) * XCOLS],
                    )
                    for half in range(2):
                        y_sb = ypool.tile(
                            [128, NCH // 2, 2, 128], y_dt, tag="ysb"
                        )
                        for g2 in range(2):
                            pm = ps_main.tile([128, GRP, 128], F32, tag="pm")
                            for k in range(GRP):
                                ck = half * 8 + g2 * GRP + k
                                nc.tensor.matmul(
                                    pm[:, k, :],
                                    xa[:, ck * 128:(ck + 1) * 128],
                                    M2blk[:, bp, :],
                                    start=True, stop=True,
                                )
                            pv = pm[:].rearrange("p k (h c) -> p k h c", h=2)
                            yv = y_sb[:, g2 * GRP:(g2 + 1) * GRP, :, :]
                            # block0=min(yl+yr,1); block1=max(yl+yr-1,0)
                            # block2=max(yl-yr,0); block3=max(yr-yl,0)
                            nc.vector.tensor_scalar_min(
                                yv[:, :, :, 0:32], pv[:, :, :, 0:32], 1.0
                            )
                            nc.vector.tensor_scalar(
                                out=yv[:, :, :, 32:64], in0=pv[:, :, :, 0:32],
                                scalar1=1.0, scalar2=0.0,
                                op0=OP.subtract, op1=OP.max,
                            )
                            nc.scalar.activation(
                                yv[:, :, :, 64:96], pv[:, :, :, 32:64], AF.Relu
                            )
                            nc.scalar.activation(
                                yv[:, :, :, 96:128], pv[:, :, :, 32:64],
                                AF.Relu, scale=-1.0,
                            )
                        p0 = t4 * XCOLS + half * (XCOLS // 2)
                        for h in range(2):
                            nc.gpsimd.dma_start(
                                y_d[2 * bp + h, p0:p0 + XCOLS // 2, :].rearrange(
                                    "(c p) f -> p c f", p=128
                                ),
                                y_sb[:, :, h, :],
                            )

            # ---- MLP (v_encode), emitted last so it fills trailing gaps ----
            w1_sb = sb.tile([128, Q // 128, L], F32)
            nc.sync.dma_start(w1_sb, w1_d.rearrange("(c p) j -> p c j", p=128))
            w2_sb = sb.tile([128, 2, L], F32)
            nc.sync.dma_start(w2_sb, w2_d.rearrange("(c p) j -> p c j", p=128))
            b1_sb = sb.tile([128, 2], F32)
            nc.sync.dma_start(b1_sb, b1_d.rearrange("(c p) -> p c", p=128))
            b2_sb = sb.tile([128, 2], F32)
            nc.sync.dma_start(b2_sb, b2_d.rearrange("(c p) -> p c", p=128))

            with tc.tile_pool(name="ps_mlp", bufs=2, space="PSUM") as ps_mlp:
                h_sb = sb.tile([128, 2, BPC], F32)
                for jc in range(2):
                    ph = ps_mlp.tile([128, BPC], F32, tag="mlp")
                    for kc in range(Q // 128):
                        nc.tensor.matmul(
                            ph[:],
                            w1_sb[:, kc, jc * 128:(jc + 1) * 128],
                            VT_mlp[:, kc, :],
                            start=(kc == 0), stop=(kc == Q // 128 - 1),
                        )
                    nc.scalar.activation(
                        h_sb[:, jc, :], ph[:], AF.Lrelu,
                        bias=b1_sb[:, jc:jc + 1], scale=1.0, alpha=0.01,
                    )
                ve_sb = sb.tile([128, 2, BPC], F32)
                for j2c in range(2):
                    pv2 = ps_mlp.tile([128, BPC], F32, tag="mlp")
                    for kc in range(2):
                        nc.tensor.matmul(
                            pv2[:],
                            w2_sb[:, kc, j2c * 128:(j2c + 1) * 128],
                            h_sb[:, kc, :],
                            start=(kc == 0), stop=(kc == 1),
                        )
                    nc.scalar.activation(
                        ve_sb[:, j2c, :], pv2[:], AF.Identity,
                        bias=b2_sb[:, j2c:j2c + 1],
                    )
                nc.sync.dma_start(
                    ve_d.rearrange("(c p) b -> p c b", p=128), ve_sb[:]
                )

    if not nc.is_finalized():
        nc.finalize()
    return nc


def kernel(x, latent_vec, gumbel_u, K_left, K_right, temp, W1, b1, W2, b2):
    global LAST_RESULTS
    x = np.ascontiguousarray(x, np.float32)
    latent_vec = np.ascontiguousarray(latent_vec, np.float32)
    gumbel_u = np.ascontiguousarray(gumbel_u, np.float32)

    # host layout prep (pure layout/dtype, no math)
    if MM_DTYPE == "bf16":
        import ml_dtypes
        xT = np.ascontiguousarray(x.transpose(0, 2, 1).astype(ml_dtypes.bfloat16))
    else:
        xT = np.ascontiguousarray(x.transpose(0, 2, 1))          # [32, 64, 8192]
    Kcat = np.ascontiguousarray(
        np.stack([K_left.transpose(0, 2, 1), K_right.transpose(0, 2, 1)], axis=1)
    ).reshape(L, Q).astype(np.float32)
    latT = np.ascontiguousarray(latent_vec.T)                     # [256, 32]
    # guT2b[(o%2)*64 + i, side*16 + o//2, b]
    g0 = gumbel_u[:, :, 0]                                        # [B, 2, 64, 32]
    guT2b = np.ascontiguousarray(
        g0.transpose(3, 2, 1, 0)                                  # [o, i, side, b]
        .reshape(16, 2, S_IN, 2, B)                               # [oh, par, i, side, b]
        .transpose(1, 2, 3, 0, 4)                                 # [par, i, side, oh, b]
        .reshape(128, 32, B)
    ).astype(np.float32)
    W1perm = np.ascontiguousarray(
        np.asarray(W1, np.float32)
        .reshape(2, S_IN, S_OUT, L).transpose(0, 2, 1, 3).reshape(Q, L)
    )
    W2c = np.ascontiguousarray(np.asarray(W2, np.float32))
    b1c = np.ascontiguousarray(np.asarray(b1, np.float32))
    b2c = np.ascontiguousarray(np.asarray(b2, np.float32))
    t_clip = float(np.clip(np.float32(np.asarray(temp).reshape(-1)[0]), EPS, 2.0))
    p_exp = 1.0 / max(t_clip, EPS)

    key = (round(p_exp, 9), MM_DTYPE, LOGITS_DTYPE, Y_DTYPE)
    if key not in _PROGRAM_CACHE:
        _PROGRAM_CACHE[key] = _build_program(p_exp)
    nc = _PROGRAM_CACHE[key]

    selc = np.zeros((128, 2), np.float32)
    selc[0:64, 0] = 1.0
    selc[64:128, 1] = 1.0
    selTc = np.ascontiguousarray(selc.T)

    in_maps = []
    for core in range(N_CORES):
        b0 = core * BPC
        in_maps.append({
            "selc": selc,
            "selTc": selTc,
            "xT": np.ascontiguousarray(xT[b0:b0 + BPC]),
            "kcat": Kcat,
            "latT": np.ascontiguousarray(latT[:, b0:b0 + BPC]),
            "guT": np.ascontiguousarray(guT2b[..., b0:b0 + BPC]),
            "w1": W1perm,
            "w2": W2c,
            "b1v": b1c,
            "b2v": b2c,
        })

    trace = bool(int(os.environ.get("CSG_TRACE", "0")))
    res = run_bass_kernel_spmd(
        nc, in_maps, core_ids=list(range(N_CORES)), trace=trace
    )
    LAST_RESULTS = res
    y = np.concatenate(
        [np.asarray(r["y"], np.float32) for r in res.results], axis=0
    )
    ve = np.concatenate([r["ve"].T for r in res.results], axis=0)
    return y.astype(np.float32), ve.astype(np.float32)
